# revision 5
# baseline (speedup 1.0000x reference)
"""Trainium2 Bass kernel for nn_Att_add_mp_norm (gnn message passing).

reference computation (B=4, N=512, D=64):
    xn     = LayerNorm(i_em) * gamma + beta                  [B,N,D]
    sq     = xn @ W_a[:D];  sk = xn @ W_a[D:]                [B,N]
    scores = LeakyReLU(sq[:,None] + sk[None,:] + b_a)        [B,N,N]
    alphas = softmax(scores, -1)[..., None]                  [B,N,N,1]
    value  = i_em[:,:,None,:] * i_em[:,None,:,:]             [B,N,N,D]
    return (alphas, value)

Sharding: 8 cores = (batch b, query-half h); each core owns 256 query rows
of one batch: writes alphas slice [256,512] and value slice [256,512,64]
(32 MiB -> output-DMA bound, the target regime).

Per-core kernel strategy:
  - value = per-feature outer products on the TensorEngine: for each d,
    matmul(lhsT=qT[d] (1x128), rhs=kT[d] (1x256)) -> PSUM [128i x 256j].
    K=1 matmuls cost only N cycles.  q/k columns are staged into 4
    "stack" strips at SBUF partitions {0,32,64,96} (tile_position row
    strips) so every matmul operand has a legal base partition.
  - PSUM evacuated in d-quads with a single rearranged-AP copy that
    d-interleaves into [128, 256j, 64d] SBUF tiles (alternating
    VectorE / ScalarE), giving fully contiguous 8 MiB output DMAs
    (64 KiB per-partition descriptors).
  - scores via a tiny K=2 matmul (ones x (sk+c) + sq x ones), LeakyReLU
    as (s*0.01) max s in one scalar_tensor_tensor, softmax with
    negated reduce_max + Exp activation with fused accum (row sum).
  - The LayerNorm affine (gamma/beta) and b_a are folded on the host into
    the attention vectors: gaq = gamma*a_q, gak = gamma*a_k,
    cc = beta.a_q + beta.a_k + b_a  (softmax-preserving scalar folded
    into the sk row before the LeakyReLU).
"""
import sys

sys.path.insert(0, "/opt/trn_rl_repo")

import numpy as np

import concourse.bass as bass
import concourse.tile as tile
from concourse import bacc
from concourse import mybir
from concourse.masks import make_identity

f32 = mybir.dt.float32

B, N, D = 4, 512, 64
QH = N // 2          # query rows per core
NCORES = 8
EPS = 1e-5
SLOPE = 0.01
NKT = N // 128       # 4 key tiles
NQT = QH // 128      # 2 query tiles


def build_program():
    nc = bacc.Bacc("TRN2", target_bir_lowering=False, debug=False)

    x = nc.dram_tensor("x", [N, D], f32, kind="ExternalInput")       # keys
    xq = nc.dram_tensor("xq", [QH, D], f32, kind="ExternalInput")    # queries
    gaq = nc.dram_tensor("gaq", [1, D], f32, kind="ExternalInput")   # gamma*a_q
    gak = nc.dram_tensor("gak", [1, D], f32, kind="ExternalInput")   # gamma*a_k
    cc = nc.dram_tensor("cc", [1, 1], f32, kind="ExternalInput")     # folded const

    alp = nc.dram_tensor("alphas", [QH, N], f32, kind="ExternalOutput")
    val = nc.dram_tensor("value", [QH, N, D], f32, kind="ExternalOutput")

    with tile.TileContext(nc) as tc:
        with (
            tc.tile_pool(name="const", bufs=1) as constp,
            tc.tile_pool(name="data", bufs=1) as datap,
            tc.tile_pool(name="stats", bufs=6) as statp,
            tc.tile_pool(name="asb", bufs=2) as asbp,
            tc.tile_pool(name="outp", bufs=2) as outp,
            tc.tile_pool(name="psT", bufs=2, space="PSUM") as psT,
            tc.tile_pool(name="psS", bufs=1, space="PSUM") as psS,
            tc.tile_pool(name="psV", bufs=2, space="PSUM") as psV,
        ):
            # ---- constants ----
            ident = constp.tile([128, 128], f32)
            make_identity(nc, ident)
            gaq_b = constp.tile([128, D], f32)
            gak_b = constp.tile([128, D], f32)
            cc_b = constp.tile([128, 1], f32)
            eps_b = constp.tile([128, 1], f32)
            nc.gpsimd.dma_start(out=gaq_b, in_=gaq.ap().to_broadcast((128, D)))
            nc.gpsimd.dma_start(out=gak_b, in_=gak.ap().to_broadcast((128, D)))
            nc.gpsimd.dma_start(out=cc_b, in_=cc.ap().to_broadcast((128, 1)))
            nc.vector.memset(eps_b, EPS)

            # ---- load inputs ----
            x_sb = datap.tile([128, NKT, D], f32)
            xq_sb = datap.tile([128, NQT, D], f32)
            nc.sync.dma_start(
                out=x_sb, in_=x.ap().rearrange("(t p) d -> p t d", p=128))
            nc.sync.dma_start(
                out=xq_sb, in_=xq.ap().rearrange("(t p) d -> p t d", p=128))

            # ---- transpose raw x / xq  ([128,64] -> [64,128] each tile) ----
            xT_sb = datap.tile([64, NKT, 128], f32)
            xqT_sb = datap.tile([64, NQT, 128], f32)
            for t in range(NKT):
                tp = psT.tile([64, 128], f32, tag="tp")
                nc.tensor.transpose(tp, x_sb[:, t, :], ident)
                nc.vector.tensor_copy(out=xT_sb[:, t, :], in_=tp)
            for t in range(NQT):
                tp = psT.tile([64, 128], f32, tag="tp")
                nc.tensor.transpose(tp, xq_sb[:, t, :], ident)
                nc.vector.tensor_copy(out=xqT_sb[:, t, :], in_=tp)

            # ---- build stacks: strip g (partition 32g) holds d=16g..16g+15
            # kstk[32g, r*512 + j] = x[j, 16g+r];  qstk[32g, r*256 + i] = xq[i, ..]
            kstk = datap.tile([128, 16 * N], f32)
            qstk = datap.tile([128, 16 * QH], f32)
            for g in range(4):
                nc.gpsimd.dma_start(
                    out=kstk[32 * g:32 * g + 1, :],
                    in_=xT_sb[16 * g:16 * g + 16, :, :])
                nc.gpsimd.dma_start(
                    out=qstk[32 * g:32 * g + 1, :],
                    in_=xqT_sb[16 * g:16 * g + 16, :, :])

            # ---- LayerNorm (no affine; folded on host) + sq/sk ----
            # cols6: 0..3 = sk per key tile (+cc), 4..5 = sq per query tile
            cols6 = datap.tile([128, NKT + NQT], f32)
            for t in range(NKT + NQT):
                src = x_sb[:, t, :] if t < NKT else xq_sb[:, t - NKT, :]
                st6 = statp.tile([128, nc.vector.BN_STATS_DIM], f32, tag="st")
                mv = statp.tile([128, nc.vector.BN_AGGR_DIM], f32, tag="mv")
                nc.vector.bn_stats(out=st6, in_=src)
                nc.vector.bn_aggr(out=mv, in_=st6)
                rstd = statp.tile([128, 1], f32, tag="rstd")
                nc.scalar.activation(
                    out=rstd, in_=mv[:, 1:2],
                    func=mybir.ActivationFunctionType.Sqrt,
                    bias=eps_b, scale=1.0)
                nc.vector.reciprocal(out=rstd, in_=rstd)
                xn = statp.tile([128, D], f32, tag="xn")
                nc.vector.tensor_scalar(
                    out=xn, in0=src, scalar1=mv[:, 0:1], scalar2=rstd,
                    op0=mybir.AluOpType.subtract, op1=mybir.AluOpType.mult)
                prod = statp.tile([128, D], f32, tag="prod")
                nc.vector.tensor_mul(
                    prod, xn, gak_b if t < NKT else gaq_b)
                nc.vector.tensor_reduce(
                    out=cols6[:, t:t + 1], in_=prod,
                    axis=mybir.AxisListType.X, op=mybir.AluOpType.add)
            # fold constant into sk columns
            nc.vector.tensor_scalar_add(cols6[:, 0:NKT], cols6[:, 0:NKT], cc_b)

            # transpose cols6 -> rows6 [6, 128]
            tp6 = psT.tile([NKT + NQT, 128], f32, tag="tp")
            nc.tensor.transpose(tp6, cols6, ident)
            rows6 = datap.tile([NKT + NQT, 128], f32)
            nc.vector.tensor_copy(out=rows6, in_=tp6)

            # scatter into matmul operand rows (partition-major flatten DMAs)
            rhs_mm = datap.tile([2, N], f32)    # row0 = ones, row1 = sk + cc
            sqrow = datap.tile([1, QH], f32)    # sq as a row
            nc.vector.memset(rhs_mm, 1.0)
            nc.gpsimd.dma_start(out=rhs_mm[1:2, :], in_=rows6[0:NKT, :])
            nc.gpsimd.dma_start(out=sqrow, in_=rows6[NKT:NKT + NQT, :])
            lhsT2 = datap.tile([2, NQT, 128], f32)  # per iblk: row0=sq, row1=1
            nc.vector.memset(lhsT2, 1.0)
            for ib in range(NQT):
                nc.vector.tensor_copy(
                    out=lhsT2[0:1, ib, :], in_=sqrow[:, ib * 128:(ib + 1) * 128])

            # ---- scores + softmax + alphas per query block ----
            for ib in range(NQT):
                sc_ps = psS.tile([128, N], f32, tag="sc")
                nc.tensor.matmul(sc_ps, lhsT2[:, ib, :], rhs_mm,
                                 start=True, stop=True)
                asb = asbp.tile([128, N], f32, tag="a")
                tmp = asbp.tile([128, N], f32, tag="t")
                # LeakyReLU: max(s, s * SLOPE); one PSUM read per op
                nc.vector.tensor_scalar_mul(tmp, sc_ps, SLOPE)
                nc.vector.tensor_max(asb, sc_ps, tmp)
                nmx = statp.tile([128, 1], f32, tag="nmx")
                nc.vector.tensor_reduce(
                    out=nmx, in_=asb, axis=mybir.AxisListType.X,
                    op=mybir.AluOpType.max, negate=True)
                ssum = statp.tile([128, 1], f32, tag="ssum")
                nc.scalar.activation(
                    out=asb, in_=asb, func=mybir.ActivationFunctionType.Exp,
                    bias=nmx, scale=1.0, accum_out=ssum)
                rinv = statp.tile([128, 1], f32, tag="rinv")
                nc.vector.reciprocal(out=rinv, in_=ssum)
                nc.vector.tensor_scalar_mul(asb, asb, rinv)
                nc.sync.dma_start(
                    out=alp.ap()[ib * 128:(ib + 1) * 128, :], in_=asb)

            # ---- value: per-d outer products ----
            for ib in range(NQT):
                for jh in range(2):
                    out_t = outp.tile([128, 256, D], f32, tag="out")
                    for dq in range(16):          # d-quads
                        ps = psV.tile([128, 4, 256], f32, tag="v")
                        for dd in range(4):
                            d = dq * 4 + dd
                            g, r = d // 16, d % 16
                            lhsT = qstk[32 * g:32 * g + 1,
                                        r * QH + ib * 128:r * QH + (ib + 1) * 128]
                            rhs = kstk[32 * g:32 * g + 1,
                                       r * N + jh * 256:r * N + (jh + 1) * 256]
                            nc.tensor.matmul(ps[:, dd, :], lhsT, rhs,
                                             start=True, stop=True,
                                             tile_position=(32 * g, 0))
                        src = ps.rearrange("p d j -> p j d")
                        dst = out_t[:, :, dq * 4:(dq + 1) * 4]
                        if dq % 2 == 0:
                            nc.vector.tensor_copy(out=dst, in_=src)
                        else:
                            nc.scalar.copy(out=dst, in_=src)
                    nc.sync.dma_start(
                        out=val.ap()[ib * 128:(ib + 1) * 128,
                                     jh * 256:(jh + 1) * 256, :],
                        in_=out_t)
    nc.compile()
    return nc


_NC_CACHE = None


def _get_nc():
    global _NC_CACHE
    if _NC_CACHE is None:
        _NC_CACHE = build_program()
    return _NC_CACHE


def make_in_maps(i_em, W_a, b_a, gamma, beta):
    i_em = np.ascontiguousarray(np.asarray(i_em, np.float32))
    W_a = np.asarray(W_a, np.float32)
    b_a = np.asarray(b_a, np.float32)
    gamma = np.asarray(gamma, np.float32)
    beta = np.asarray(beta, np.float32)
    aq, ak = W_a[:D], W_a[D:]
    gaq = (gamma * aq).reshape(1, D)
    gak = (gamma * ak).reshape(1, D)
    cc = np.float32(beta @ aq + beta @ ak + b_a[0]).reshape(1, 1)
    maps = []
    for c in range(NCORES):
        b, h = c // 2, c % 2
        maps.append({
            "x": i_em[b],
            "xq": np.ascontiguousarray(i_em[b, h * QH:(h + 1) * QH]),
            "gaq": gaq, "gak": gak, "cc": cc,
        })
    return maps


def assemble(results):
    alphas = np.empty((B, N, N, 1), np.float32)
    value = np.empty((B, N, N, D), np.float32)
    for c in range(NCORES):
        b, h = c // 2, c % 2
        alphas[b, h * QH:(h + 1) * QH, :, 0] = results[c]["alphas"]
        value[b, h * QH:(h + 1) * QH] = results[c]["value"]
    return alphas, value


def kernel(i_em, W_a, b_a, gamma, beta):
    from concourse.bass_utils import run_bass_kernel_spmd
    nc = _get_nc()
    in_maps = make_in_maps(i_em, W_a, b_a, gamma, beta)
    res = run_bass_kernel_spmd(nc, in_maps, list(range(NCORES)))
    return assemble(res.results)


# revision 7
# speedup vs baseline: 6339.6603x; 6339.6603x over previous
"""Trainium2 Bass kernel for nn_Att_add_mp_norm (gnn message passing).

reference computation (B=4, N=512, D=64):
    xn     = LayerNorm(i_em) * gamma + beta                  [B,N,D]
    sq     = xn @ W_a[:D];  sk = xn @ W_a[D:]                [B,N]
    scores = LeakyReLU(sq[:,None] + sk[None,:] + b_a)        [B,N,N]
    alphas = softmax(scores, -1)[..., None]                  [B,N,N,1]
    value  = i_em[:,:,None,:] * i_em[:,None,:,:]             [B,N,N,D]
    return (alphas, value)

Sharding: 8 cores = (batch b, query-half h); each core owns 256 query rows
of one batch: writes alphas slice [256,512] and value slice [256,512,64]
(32 MiB -> output-DMA bound, the target regime).

Per-core kernel strategy:
  - value = per-feature outer products on the TensorEngine: for each d,
    matmul(lhsT=qT[d] (1x128), rhs=kT[d] (1x256)) -> PSUM [128i x 256j].
    K=1 matmuls cost only N cycles.  q/k columns are staged into 4
    "stack" strips at SBUF partitions {0,32,64,96} (tile_position row
    strips) so every matmul operand has a legal base partition.
  - PSUM evacuated in d-quads with a single rearranged-AP copy that
    d-interleaves into [128, 256j, 64d] SBUF tiles (alternating
    VectorE / ScalarE), giving fully contiguous 8 MiB output DMAs
    (64 KiB per-partition descriptors).
  - scores via a tiny K=2 matmul (ones x (sk+c) + sq x ones), LeakyReLU
    as (s*0.01) max s in one scalar_tensor_tensor, softmax with
    negated reduce_max + Exp activation with fused accum (row sum).
  - The LayerNorm affine (gamma/beta) and b_a are folded on the host into
    the attention vectors: gaq = gamma*a_q, gak = gamma*a_k,
    cc = beta.a_q + beta.a_k + b_a  (softmax-preserving scalar folded
    into the sk row before the LeakyReLU).
"""
import sys

sys.path.insert(0, "/opt/trn_rl_repo")

import numpy as np

import concourse.bass as bass
import concourse.tile as tile
from concourse import bacc
from concourse import mybir
from concourse.masks import make_identity

f32 = mybir.dt.float32

B, N, D = 4, 512, 64
QH = N // 2          # query rows per core
NCORES = 8
EPS = 1e-5
SLOPE = 0.01
NKT = N // 128       # 4 key tiles
NQT = QH // 128      # 2 query tiles


def build_program(reps=None):
    """reps=None: normal grading program (value is an external output).
    reps=R: timing variant — value is an internal DRAM tensor (not
    transferred off-device) and the whole body runs R times inside a
    dynamic For_i loop so on-device time can be measured differentially."""
    nc = bacc.Bacc("TRN2", target_bir_lowering=False, debug=False)

    x = nc.dram_tensor("x", [N, D], f32, kind="ExternalInput")       # keys
    xq = nc.dram_tensor("xq", [QH, D], f32, kind="ExternalInput")    # queries
    gaq = nc.dram_tensor("gaq", [1, D], f32, kind="ExternalInput")   # gamma*a_q
    gak = nc.dram_tensor("gak", [1, D], f32, kind="ExternalInput")   # gamma*a_k
    cc = nc.dram_tensor("cc", [1, 1], f32, kind="ExternalInput")     # folded const

    alp = nc.dram_tensor("alphas", [QH, N], f32, kind="ExternalOutput")
    if reps is None:
        val = nc.dram_tensor("value", [QH, N, D], f32, kind="ExternalOutput")
    else:
        val = nc.dram_tensor("value", [QH, N, D], f32)

    from contextlib import ExitStack
    with tile.TileContext(nc) as tc, ExitStack() as stk:
        with (
            tc.tile_pool(name="const", bufs=1) as constp,
            tc.tile_pool(name="data", bufs=1) as datap,
            tc.tile_pool(name="stats", bufs=6) as statp,
            tc.tile_pool(name="asb", bufs=2) as asbp,
            tc.tile_pool(name="outp", bufs=2) as outp,
            tc.tile_pool(name="psT", bufs=2, space="PSUM") as psT,
            tc.tile_pool(name="psS", bufs=1, space="PSUM") as psS,
            tc.tile_pool(name="psV", bufs=2, space="PSUM") as psV,
        ):
            if reps is not None:
                stk.enter_context(tc.For_i(0, reps, 1))
            # ---- constants ----
            ident = constp.tile([128, 128], f32)
            make_identity(nc, ident)
            gaq_b = constp.tile([128, D], f32)
            gak_b = constp.tile([128, D], f32)
            cc_b = constp.tile([128, 1], f32)
            eps_b = constp.tile([128, 1], f32)
            nc.gpsimd.dma_start(out=gaq_b, in_=gaq.ap().to_broadcast((128, D)))
            nc.gpsimd.dma_start(out=gak_b, in_=gak.ap().to_broadcast((128, D)))
            nc.gpsimd.dma_start(out=cc_b, in_=cc.ap().to_broadcast((128, 1)))
            nc.vector.memset(eps_b, EPS)

            # ---- load inputs ----
            x_sb = datap.tile([128, NKT, D], f32)
            xq_sb = datap.tile([128, NQT, D], f32)
            nc.sync.dma_start(
                out=x_sb, in_=x.ap().rearrange("(t p) d -> p t d", p=128))
            nc.sync.dma_start(
                out=xq_sb, in_=xq.ap().rearrange("(t p) d -> p t d", p=128))

            # ---- transpose raw x / xq  ([128,64] -> [64,128] each tile) ----
            xT_sb = datap.tile([64, NKT, 128], f32)
            xqT_sb = datap.tile([64, NQT, 128], f32)
            for t in range(NKT):
                tp = psT.tile([64, 128], f32, tag="tp")
                nc.tensor.transpose(tp, x_sb[:, t, :], ident)
                nc.vector.tensor_copy(out=xT_sb[:, t, :], in_=tp)
            for t in range(NQT):
                tp = psT.tile([64, 128], f32, tag="tp")
                nc.tensor.transpose(tp, xq_sb[:, t, :], ident)
                nc.vector.tensor_copy(out=xqT_sb[:, t, :], in_=tp)

            # ---- build stacks: strip g (partition 32g) holds d=16g..16g+15
            # kstk[32g, r*512 + j] = x[j, 16g+r];  qstk[32g, r*256 + i] = xq[i, ..]
            kstk = datap.tile([128, 16 * N], f32)
            qstk = datap.tile([128, 16 * QH], f32)
            for g in range(4):
                nc.gpsimd.dma_start(
                    out=kstk[32 * g:32 * g + 1, :],
                    in_=xT_sb[16 * g:16 * g + 16, :, :])
                nc.gpsimd.dma_start(
                    out=qstk[32 * g:32 * g + 1, :],
                    in_=xqT_sb[16 * g:16 * g + 16, :, :])

            # ---- LayerNorm (no affine; folded on host) + sq/sk ----
            # cols6: 0..3 = sk per key tile (+cc), 4..5 = sq per query tile
            cols6 = datap.tile([128, NKT + NQT], f32)
            for t in range(NKT + NQT):
                src = x_sb[:, t, :] if t < NKT else xq_sb[:, t - NKT, :]
                st6 = statp.tile([128, nc.vector.BN_STATS_DIM], f32, tag="st")
                mv = statp.tile([128, nc.vector.BN_AGGR_DIM], f32, tag="mv")
                nc.vector.bn_stats(out=st6, in_=src)
                nc.vector.bn_aggr(out=mv, in_=st6)
                rstd = statp.tile([128, 1], f32, tag="rstd")
                nc.scalar.activation(
                    out=rstd, in_=mv[:, 1:2],
                    func=mybir.ActivationFunctionType.Sqrt,
                    bias=eps_b, scale=1.0)
                nc.vector.reciprocal(out=rstd, in_=rstd)
                xn = statp.tile([128, D], f32, tag="xn")
                nc.vector.tensor_scalar(
                    out=xn, in0=src, scalar1=mv[:, 0:1], scalar2=rstd,
                    op0=mybir.AluOpType.subtract, op1=mybir.AluOpType.mult)
                prod = statp.tile([128, D], f32, tag="prod")
                nc.vector.tensor_mul(
                    prod, xn, gak_b if t < NKT else gaq_b)
                nc.vector.tensor_reduce(
                    out=cols6[:, t:t + 1], in_=prod,
                    axis=mybir.AxisListType.X, op=mybir.AluOpType.add)
            # fold constant into sk columns
            nc.vector.tensor_scalar_add(cols6[:, 0:NKT], cols6[:, 0:NKT], cc_b)

            # transpose cols6 -> rows6 [6, 128]
            tp6 = psT.tile([NKT + NQT, 128], f32, tag="tp")
            nc.tensor.transpose(tp6, cols6, ident)
            rows6 = datap.tile([NKT + NQT, 128], f32)
            nc.vector.tensor_copy(out=rows6, in_=tp6)

            # scatter into matmul operand rows (partition-major flatten DMAs)
            rhs_mm = datap.tile([2, N], f32)    # row0 = ones, row1 = sk + cc
            sqrow = datap.tile([1, QH], f32)    # sq as a row
            nc.vector.memset(rhs_mm, 1.0)
            nc.gpsimd.dma_start(out=rhs_mm[1:2, :], in_=rows6[0:NKT, :])
            nc.gpsimd.dma_start(out=sqrow, in_=rows6[NKT:NKT + NQT, :])
            lhsT2 = datap.tile([2, NQT, 128], f32)  # per iblk: row0=sq, row1=1
            nc.vector.memset(lhsT2, 1.0)
            for ib in range(NQT):
                nc.vector.tensor_copy(
                    out=lhsT2[0:1, ib, :], in_=sqrow[:, ib * 128:(ib + 1) * 128])

            # ---- scores + softmax + alphas per query block ----
            for ib in range(NQT):
                sc_ps = psS.tile([128, N], f32, tag="sc")
                nc.tensor.matmul(sc_ps, lhsT2[:, ib, :], rhs_mm,
                                 start=True, stop=True)
                asb = asbp.tile([128, N], f32, tag="a")
                tmp = asbp.tile([128, N], f32, tag="t")
                # LeakyReLU: max(s, s * SLOPE); one PSUM read per op
                nc.vector.tensor_scalar_mul(tmp, sc_ps, SLOPE)
                nc.vector.tensor_max(asb, sc_ps, tmp)
                nmx = statp.tile([128, 1], f32, tag="nmx")
                nc.vector.tensor_reduce(
                    out=nmx, in_=asb, axis=mybir.AxisListType.X,
                    op=mybir.AluOpType.max, negate=True)
                ssum = statp.tile([128, 1], f32, tag="ssum")
                nc.scalar.activation(
                    out=asb, in_=asb, func=mybir.ActivationFunctionType.Exp,
                    bias=nmx, scale=1.0, accum_out=ssum)
                rinv = statp.tile([128, 1], f32, tag="rinv")
                nc.vector.reciprocal(out=rinv, in_=ssum)
                nc.vector.tensor_scalar_mul(asb, asb, rinv)
                nc.sync.dma_start(
                    out=alp.ap()[ib * 128:(ib + 1) * 128, :], in_=asb)

            # ---- value: per-d outer products ----
            for ib in range(NQT):
                for jh in range(2):
                    out_t = outp.tile([128, 256, D], f32, tag="out")
                    for dq in range(16):          # d-quads
                        ps = psV.tile([128, 4, 256], f32, tag="v")
                        for dd in range(4):
                            d = dq * 4 + dd
                            g, r = d // 16, d % 16
                            lhsT = qstk[32 * g:32 * g + 1,
                                        r * QH + ib * 128:r * QH + (ib + 1) * 128]
                            rhs = kstk[32 * g:32 * g + 1,
                                       r * N + jh * 256:r * N + (jh + 1) * 256]
                            nc.tensor.matmul(ps[:, dd, :], lhsT, rhs,
                                             start=True, stop=True,
                                             tile_position=(32 * g, 0))
                        src = ps.rearrange("p d j -> p j d")
                        dst = out_t[:, :, dq * 4:(dq + 1) * 4]
                        if dq % 2 == 0:
                            nc.vector.tensor_copy(out=dst, in_=src)
                        else:
                            nc.scalar.copy(out=dst, in_=src)
                    nc.sync.dma_start(
                        out=val.ap()[ib * 128:(ib + 1) * 128,
                                     jh * 256:(jh + 1) * 256, :],
                        in_=out_t)
            stk.close()
    nc.compile()
    return nc


_NC_CACHE = {}


def _get_nc(reps=None):
    if reps not in _NC_CACHE:
        _NC_CACHE[reps] = build_program(reps)
    return _NC_CACHE[reps]


def make_in_maps(i_em, W_a, b_a, gamma, beta):
    i_em = np.ascontiguousarray(np.asarray(i_em, np.float32))
    W_a = np.asarray(W_a, np.float32)
    b_a = np.asarray(b_a, np.float32)
    gamma = np.asarray(gamma, np.float32)
    beta = np.asarray(beta, np.float32)
    aq, ak = W_a[:D], W_a[D:]
    gaq = (gamma * aq).reshape(1, D)
    gak = (gamma * ak).reshape(1, D)
    cc = np.float32(beta @ aq + beta @ ak + b_a[0]).reshape(1, 1)
    maps = []
    for c in range(NCORES):
        b, h = c // 2, c % 2
        maps.append({
            "x": i_em[b],
            "xq": np.ascontiguousarray(i_em[b, h * QH:(h + 1) * QH]),
            "gaq": gaq, "gak": gak, "cc": cc,
        })
    return maps


def assemble(results):
    alphas = np.empty((B, N, N, 1), np.float32)
    value = np.empty((B, N, N, D), np.float32)
    for c in range(NCORES):
        b, h = c // 2, c % 2
        alphas[b, h * QH:(h + 1) * QH, :, 0] = results[c]["alphas"]
        value[b, h * QH:(h + 1) * QH] = results[c]["value"]
    return alphas, value


def kernel(i_em, W_a, b_a, gamma, beta):
    from concourse.bass_utils import run_bass_kernel_spmd
    nc = _get_nc()
    in_maps = make_in_maps(i_em, W_a, b_a, gamma, beta)
    res = run_bass_kernel_spmd(nc, in_maps, list(range(NCORES)))
    return assemble(res.results)


# revision 20
# speedup vs baseline: 68007.2551x; 10.7273x over previous
"""Trainium2 Bass kernel for nn_Att_add_mp_norm (gnn message passing).

reference computation (B=4, N=512, D=64):
    xn     = LayerNorm(i_em) * gamma + beta                  [B,N,D]
    sq     = xn @ W_a[:D];  sk = xn @ W_a[D:]                [B,N]
    scores = LeakyReLU(sq[:,None] + sk[None,:] + b_a)        [B,N,N]
    alphas = softmax(scores, -1)[..., None]                  [B,N,N,1]
    value  = i_em[:,:,None,:] * i_em[:,None,:,:]             [B,N,N,D]
    return (alphas, value)

Sharding: 8 cores = (batch b, query-half h); each core owns 256 query rows
of one batch: writes alphas slice [256,512] and value slice [256,512,64]
(32 MiB -> output-DMA bound, the target regime).

Per-core kernel strategy:
  - value = per-feature outer products on the TensorEngine: for each d,
    matmul(lhsT=qT[d] (1x128), rhs=kT[d] (1x256)) -> PSUM [128i x 256j].
    K=1 matmuls cost only N cycles.  q/k columns are staged into 4
    "stack" strips at SBUF partitions {0,32,64,96} (tile_position row
    strips) so every matmul operand has a legal base partition.
  - PSUM evacuated in d-quads with a single rearranged-AP copy that
    d-interleaves into [128, 256j, 64d] SBUF tiles (alternating
    VectorE / ScalarE), giving fully contiguous 8 MiB output DMAs
    (64 KiB per-partition descriptors).
  - scores via a tiny K=2 matmul (ones x (sk+c) + sq x ones), LeakyReLU
    as (s*0.01) max s in one scalar_tensor_tensor, softmax with
    negated reduce_max + Exp activation with fused accum (row sum).
  - The LayerNorm affine (gamma/beta) and b_a are folded on the host into
    the attention vectors: gaq = gamma*a_q, gak = gamma*a_k,
    cc = beta.a_q + beta.a_k + b_a  (softmax-preserving scalar folded
    into the sk row before the LeakyReLU).
"""
import sys

sys.path.insert(0, "/opt/trn_rl_repo")

import numpy as np

import concourse.bass as bass
import concourse.tile as tile
from concourse import bacc
from concourse import mybir
from concourse.masks import make_identity

f32 = mybir.dt.float32

B, N, D = 4, 512, 64
QH = N // 2          # query rows per core
NCORES = 8
EPS = 1e-5
SLOPE = 0.01
NKT = N // 128       # 4 key tiles
NQT = QH // 128      # 2 query tiles


def build_program(reps=None, cut=None):
    """reps=None: normal grading program (value is an external output).
    reps=R: timing variant — value is an internal DRAM tensor (not
    transferred off-device) and the whole body runs R times inside a
    dynamic For_i loop so on-device time can be measured differentially.
    cut: None | 'nodma' (skip value output DMAs) | 'nocopy' (also skip
    PSUM evacuation copies) | 'novalue' (skip the whole value loop)."""
    nc = bacc.Bacc("TRN2", target_bir_lowering=False, debug=False)

    x = nc.dram_tensor("x", [128, NKT, D], f32, kind="ExternalInput")    # keys, pre-tiled
    xq = nc.dram_tensor("xq", [128, NQT, D], f32, kind="ExternalInput")  # queries, pre-tiled
    gaq = nc.dram_tensor("gaq", [128, D], f32, kind="ExternalInput")   # gamma*a_q bcast
    gak = nc.dram_tensor("gak", [128, D], f32, kind="ExternalInput")   # gamma*a_k bcast
    cc = nc.dram_tensor("cc", [128, 1], f32, kind="ExternalInput")     # folded const bcast

    if reps is None:
        alp = nc.dram_tensor("alphas", [QH, N], f32, kind="ExternalOutput")
        val = nc.dram_tensor("value", [QH, N, D], f32, kind="ExternalOutput")
        chk = None
    else:
        alp = nc.dram_tensor("alphas", [QH, N], f32)
        val = nc.dram_tensor("value", [QH, N, D], f32)
        chk = nc.dram_tensor("chk", [1, 1], f32, kind="ExternalOutput")

    from contextlib import ExitStack
    with tile.TileContext(nc) as tc, ExitStack() as stk:
        with (
            tc.tile_pool(name="const", bufs=1) as constp,
            tc.tile_pool(name="data", bufs=1) as datap,
            tc.tile_pool(name="stats", bufs=6) as statp,
            tc.tile_pool(name="asb", bufs=2) as asbp,
            tc.tile_pool(name="outp", bufs=2) as outp,
            tc.tile_pool(name="psV", bufs=2, space="PSUM") as psV,
        ):
            if reps is not None:
                stk.enter_context(tc.For_i(0, reps, 1))
            # ---- constants ----
            ident = constp.tile([128, 128], f32)
            make_identity(nc, ident)
            gaq_b = constp.tile([128, D], f32)
            gak_b = constp.tile([128, D], f32)
            cc_b = constp.tile([128, 1], f32)
            eps_b = constp.tile([128, 1], f32)
            nc.sync.dma_start(out=gaq_b, in_=gaq.ap())
            nc.sync.dma_start(out=gak_b, in_=gak.ap())
            nc.sync.dma_start(out=cc_b, in_=cc.ap())
            nc.vector.memset(eps_b, EPS)

            # ---- load inputs ----
            x_sb = datap.tile([128, NKT, D], f32)
            xq_sb = datap.tile([128, NQT, D], f32)
            nc.sync.dma_start(out=x_sb, in_=x.ap())
            nc.sync.dma_start(out=xq_sb, in_=xq.ap())

            # ---- transpose raw x / xq  ([128,64] -> [64,128] each tile) ----
            xT_sb = datap.tile([64, NKT, 128], f32)
            xqT_sb = datap.tile([64, NQT, 128], f32)
            for t in range(NKT):
                tp = psV.tile([64, 128], f32, tag="v")
                nc.tensor.transpose(tp, x_sb[:, t, :], ident)
                nc.vector.tensor_copy(out=xT_sb[:, t, :], in_=tp)
            for t in range(NQT):
                tp = psV.tile([64, 128], f32, tag="v")
                nc.tensor.transpose(tp, xq_sb[:, t, :], ident)
                nc.vector.tensor_copy(out=xqT_sb[:, t, :], in_=tp)

            # ---- build stacks: strip g (partition 32g) holds d=16g..16g+15
            # kstk[32g, r*512 + j] = x[j, 16g+r];  qstk[32g, r*256 + i] = xq[i, ..]
            kstk = datap.tile([128, 16 * N], f32)
            qstk = datap.tile([128, 16 * QH], f32)
            for g in range(4):
                nc.sync.dma_start(
                    out=kstk[32 * g:32 * g + 1, :],
                    in_=xT_sb[16 * g:16 * g + 16, :, :])
                nc.sync.dma_start(
                    out=qstk[32 * g:32 * g + 1, :],
                    in_=xqT_sb[16 * g:16 * g + 16, :, :])

            # ---- LayerNorm (no affine; folded on host) + sq/sk ----
            # cols6: 0..3 = sk per key tile (+cc), 4..5 = sq per query tile
            cols6 = datap.tile([128, NKT + NQT], f32)
            for t in range(NKT + NQT):
                src = x_sb[:, t, :] if t < NKT else xq_sb[:, t - NKT, :]
                st6 = statp.tile([128, nc.vector.BN_STATS_DIM], f32, tag="st")
                mv = statp.tile([128, nc.vector.BN_AGGR_DIM], f32, tag="mv")
                nc.vector.bn_stats(out=st6, in_=src)
                nc.vector.bn_aggr(out=mv, in_=st6)
                rstd = statp.tile([128, 1], f32, tag="rstd")
                nc.scalar.activation(
                    out=rstd, in_=mv[:, 1:2],
                    func=mybir.ActivationFunctionType.Sqrt,
                    bias=eps_b, scale=1.0)
                nc.vector.reciprocal(out=rstd, in_=rstd)
                xn = statp.tile([128, D], f32, tag="xn")
                nc.vector.tensor_scalar(
                    out=xn, in0=src, scalar1=mv[:, 0:1], scalar2=rstd,
                    op0=mybir.AluOpType.subtract, op1=mybir.AluOpType.mult)
                prod = statp.tile([128, D], f32, tag="prod")
                nc.vector.tensor_mul(
                    prod, xn, gak_b if t < NKT else gaq_b)
                nc.vector.tensor_reduce(
                    out=cols6[:, t:t + 1], in_=prod,
                    axis=mybir.AxisListType.X, op=mybir.AluOpType.add)
            # fold constant into sk columns
            nc.vector.tensor_scalar_add(cols6[:, 0:NKT], cols6[:, 0:NKT], cc_b)

            # transpose cols6 -> rows6 [6, 128]
            tp6 = psV.tile([NKT + NQT, 128], f32, tag="v")
            nc.tensor.transpose(tp6, cols6, ident)
            rows6 = datap.tile([NKT + NQT, 128], f32)
            nc.vector.tensor_copy(out=rows6, in_=tp6)

            # scatter into matmul operand rows (partition-major flatten DMAs)
            rhs_mm = datap.tile([2, N], f32)    # row0 = ones, row1 = sk + cc
            sqrow = datap.tile([1, QH], f32)    # sq as a row
            nc.vector.memset(rhs_mm, 1.0)
            nc.gpsimd.dma_start(out=rhs_mm[1:2, :], in_=rows6[0:NKT, :])
            nc.gpsimd.dma_start(out=sqrow, in_=rows6[NKT:NKT + NQT, :])
            lhsT2 = datap.tile([2, NQT, 128], f32)  # per iblk: row0=sq, row1=1
            nc.vector.memset(lhsT2, 1.0)
            for ib in range(NQT):
                nc.vector.tensor_copy(
                    out=lhsT2[0:1, ib, :], in_=sqrow[:, ib * 128:(ib + 1) * 128])

            # ---- scores + softmax + alphas per query block ----
            for ib in range(NQT):
                sc_ps = psV.tile([128, N], f32, tag="v")
                nc.tensor.matmul(sc_ps, lhsT2[:, ib, :], rhs_mm,
                                 start=True, stop=True)
                asb = asbp.tile([128, N], f32, tag="a")
                tmp = asbp.tile([128, N], f32, tag="t")
                # LeakyReLU: max(s, s * SLOPE); one PSUM read per op
                nc.vector.tensor_scalar_mul(tmp, sc_ps, SLOPE)
                nc.vector.tensor_max(asb, sc_ps, tmp)
                nmx = statp.tile([128, 1], f32, tag="nmx")
                nc.vector.tensor_reduce(
                    out=nmx, in_=asb, axis=mybir.AxisListType.X,
                    op=mybir.AluOpType.max, negate=True)
                ssum = statp.tile([128, 1], f32, tag="ssum")
                nc.scalar.activation(
                    out=asb, in_=asb, func=mybir.ActivationFunctionType.Exp,
                    bias=nmx, scale=1.0, accum_out=ssum)
                rinv = statp.tile([128, 1], f32, tag="rinv")
                nc.vector.reciprocal(out=rinv, in_=ssum)
                nc.vector.tensor_scalar_mul(asb, asb, rinv)
                nc.sync.dma_start(
                    out=alp.ap()[ib * 128:(ib + 1) * 128, :], in_=asb)

            # ---- value: per-d outer products ----
            for ib in range(NQT if cut != 'novalue' else 0):
                for jh in range(2):
                    out_t = outp.tile([128, 256, D], f32, tag="out")
                    for dq in range(16):          # d-quads
                        ps = psV.tile([128, 4, 512], f32, tag="v")
                        # slot dd holds d = 16*dd + dq: the 4 matmuls hit 4
                        # different PE row-strips -> run concurrently
                        for dd in range(4):
                            g, r = dd, dq
                            lhsT = qstk[32 * g:32 * g + 1,
                                        r * QH + ib * 128:r * QH + (ib + 1) * 128]
                            rhs = kstk[32 * g:32 * g + 1,
                                       r * N + jh * 256:r * N + (jh + 1) * 256]
                            nc.tensor.matmul(ps[:, dd, 0:256], lhsT, rhs,
                                             start=True, stop=True,
                                             tile_position=(32 * g, 0))
                        if cut == 'nocopy':
                            continue
                        if cut == 'contig':
                            nc.vector.tensor_copy(
                                out=out_t[:, (dq % 4) * 16:(dq % 4) * 16 + 16, :],
                                in_=ps[:, :, 0:256])
                            continue
                        src = ps.rearrange("p d j -> p j d")
                        dst = out_t.rearrange(
                            "p j (g r) -> p j r g", r=16)[:, :, dq, :]
                        nc.vector.tensor_copy(
                            out=dst[:, 0:96, :], in_=src[:, 0:96, :])
                        nc.scalar.copy(
                            out=dst[:, 96:256, :], in_=src[:, 96:256, :])
                    if cut is None:
                        nc.sync.dma_start(
                            out=val.ap()[ib * 128:(ib + 1) * 128,
                                         jh * 256:(jh + 1) * 256, :],
                            in_=out_t)
            stk.close()
            if chk is not None:
                nc.sync.dma_start(out=chk.ap(), in_=cc_b[0:1, 0:1])
    nc.compile()
    return nc


_NC_CACHE = {}


def _get_nc(reps=None, cut=None):
    key = (reps, cut)
    if key not in _NC_CACHE:
        _NC_CACHE[key] = build_program(reps, cut)
    return _NC_CACHE[key]


def make_in_maps(i_em, W_a, b_a, gamma, beta):
    i_em = np.ascontiguousarray(np.asarray(i_em, np.float32))
    W_a = np.asarray(W_a, np.float32)
    b_a = np.asarray(b_a, np.float32)
    gamma = np.asarray(gamma, np.float32)
    beta = np.asarray(beta, np.float32)
    aq, ak = W_a[:D], W_a[D:]
    gaq = (gamma * aq).reshape(1, D)
    gak = (gamma * ak).reshape(1, D)
    cc = np.float32(beta @ aq + beta @ ak + b_a[0]).reshape(1, 1)
    gaq_b = np.ascontiguousarray(np.broadcast_to(gaq, (128, D)))
    gak_b = np.ascontiguousarray(np.broadcast_to(gak, (128, D)))
    cc_b = np.ascontiguousarray(np.broadcast_to(cc, (128, 1)))
    maps = []
    for c in range(NCORES):
        b, h = c // 2, c % 2
        xt = np.ascontiguousarray(
            i_em[b].reshape(NKT, 128, D).transpose(1, 0, 2))
        xqt = np.ascontiguousarray(
            i_em[b, h * QH:(h + 1) * QH].reshape(NQT, 128, D).transpose(1, 0, 2))
        maps.append({"x": xt, "xq": xqt, "gaq": gaq_b, "gak": gak_b,
                     "cc": cc_b})
    return maps


def assemble(results):
    alphas = np.empty((B, N, N, 1), np.float32)
    value = np.empty((B, N, N, D), np.float32)
    for c in range(NCORES):
        b, h = c // 2, c % 2
        alphas[b, h * QH:(h + 1) * QH, :, 0] = results[c]["alphas"]
        value[b, h * QH:(h + 1) * QH] = results[c]["value"]
    return alphas, value


def kernel(i_em, W_a, b_a, gamma, beta):
    from concourse.bass_utils import run_bass_kernel_spmd
    nc = _get_nc()
    in_maps = make_in_maps(i_em, W_a, b_a, gamma, beta)
    res = run_bass_kernel_spmd(nc, in_maps, list(range(NCORES)))
    return assemble(res.results)
